# revision 36
# baseline (speedup 1.0000x reference)
"""Trainium2 Bass kernel for nn_DifferentialAttention (sparse attention).

Reference computation (per batch element b):
    Q = x @ Wq + bq ; K = x @ Wk + bk ; V = x @ Wv + bv        [S, KD]
    scores  = Q @ K^T                                          [S, S]
    weights = softmax(scores, axis=-1)
    mask    = weights > mean(weights, axis=-1, keepdims=True)
    out     = (weights * mask) @ V                             [S, KD]

Key identities:
  * mean(softmax row) == 1/S exactly, so the mask is  w_ij > 1/S,
    i.e.  e_ij > den_i / S  with  e_ij = exp(s_ij - M),
    den_i = sum_j e_ij  (the global shift M cancels).
  * out_i = (sum_j g_ij V_j) / (sum_j g_ij)  with g = e * [e > den/S]:
    the normalization makes the result exactly invariant to any per-row
    scale error, so e only needs ~1e-3 relative accuracy (fp16 scores,
    bf16 e storage).

Single-pass structure (scores computed ONCE, exp computed ONCE):
  A) per 128-key chunk jc: s = K_jc^T Q (fp16 matmul, [j,i] layout),
     e = exp(s - M) -> bf16 SBUF pair-tiles [128, 2048] (16 MB total),
     den accumulated on the PE via ones-column matmuls per chunk.
  B) t = broadcast(den/S) via a 1-row matmul, m = (e > t) and g = m*e on
     the DVE (2x bf16 mode), outT[65, i] += V_aug_jc^T g  where V_aug
     carries a ones column so row 64 accumulates rho = sum g for free.
  Tail: PE-transpose outT per 128-row chunk, scale by 1/rho on the ACT
  (Copy with per-partition scale), stage into o_all, ONE batched DMA out.

Scheduling: the two query-column halves are software-pipelined (phase B
of half 0 overlaps phase A of half 1), and with repeat>1 each body's
B1-phase + tails are emitted inside the NEXT body's A-windows so the
activation engine streams exps nearly continuously.  K-projection blocks
and the threshold chains are interleaved into the A0/A1 loops to avoid
head-blocking the in-order engine queues.  DMas are batched (packed
Wqkv, 4 xT loads, one rearranged-AP output store) because each dma_start
costs ~2us of descriptor-issue time.

Sharding: 8 cores = (batch b in 0..3) x (query-row half h in 0..1).
Each core computes out[b, h*2048:(h+1)*2048, :].  The host feeds each
core x[b]^T (fp16) with columns rotated so the core's own rows come
first; row order of K/V is softmax-invariant.
"""

import os
import sys

for _p in ("/opt/trn_rl_repo", "/opt/pypackages"):
    if _p not in sys.path and os.path.isdir(_p):
        sys.path.append(_p)

import numpy as np

import concourse.bass as bass
import concourse.tile as tile
from concourse import bacc, mybir

F32 = mybir.dt.float32
F32R = mybir.dt.float32r
F16 = mybir.dt.float16
BF16 = mybir.dt.bfloat16
EXP = mybir.ActivationFunctionType.Exp
COPY = mybir.ActivationFunctionType.Copy
IDENT_FN = mybir.ActivationFunctionType.Identity
ADD = mybir.AluOpType.add
MULT = mybir.AluOpType.mult
IS_GT = mybir.AluOpType.is_gt

B, S, D, KD = 4, 4096, 256, 64
KDA = KD + 1             # V augmented with a ones column (rho accumulator)
NCORES = 8
HALF = S // 2            # query rows per core (2048)
IW = 1024                # query columns per software-pipeline half
NJC = S // 128           # 32 key chunks of 128
M_SHIFT = 30.0           # keeps exp(s - M) comfortably inside f32/bf16 range


def build_program(repeat: int = 1, phase: int = 4) -> bass.Bass:
    """repeat>1 builds the same kernel body N times back-to-back (timing aid).

    phase: 1=projections only, 2=+phase A half0, 3=+half1/B0, 4=full.
    With phase=4 and repeat>1, each body's B1+tails are software-pipelined
    into the NEXT body's A-phase windows (fills the ACT idle gap).
    """
    nc = bacc.Bacc("TRN2", target_bir_lowering=False, debug=False)

    WPACK = KD + KD + KDA    # Wq | Wk | Wv_aug columns
    xT_d = nc.dram_tensor("xT", [D, S], F16, kind="ExternalInput")
    wqkv_d = nc.dram_tensor("Wqkv", [D, WPACK], F16, kind="ExternalInput")
    bqk_d = nc.dram_tensor("bqk_col", [KD, 2], F32, kind="ExternalInput")
    bvr_d = nc.dram_tensor("bv_aug_row", [1, KDA], F16, kind="ExternalInput")
    out_d = nc.dram_tensor("out", [HALF, KD], F32, kind="ExternalOutput")

    ident_d = nc.inline_tensor(np.eye(128, dtype=np.float32), name="ident")

    with tile.TileContext(nc) as tc:
        with (
            tc.tile_pool(name="const", bufs=1) as cst,
            tc.tile_pool(name="work", bufs=3) as work,
            tc.tile_pool(name="sc", bufs=2 if "pe" in DEN_MODE else 3,
                         space="PSUM") as scp,
            tc.tile_pool(name="pb", bufs=1, space="PSUM") as pbp,
            tc.tile_pool(name="dn", bufs=1, space="PSUM") as dnp,
        ):
            # ---- persistent SBUF tiles (tags reused across repeats) ------
            xt = [cst.tile([128, S], F16, name=f"xt{dc}", tag=f"xt{dc}")
                  for dc in range(2)]

            bvb = cst.tile([128, 4 * KDA], BF16, tag="bvb")
            v_all = cst.tile([128, NJC * KDA], BF16, tag="v_all")
            v_sb = [v_all[:, jc * KDA:(jc + 1) * KDA] for jc in range(NJC)]
            ep = [[cst.tile([128, 2 * IW], BF16, name=f"ep{p}_{ih}",
                            tag=f"ep{p}_{ih}") for ih in range(2)]
                  for p in range(NPAIR)]
            t_sb = [cst.tile([128, IW], BF16, name=f"t_sb{ih}",
                             tag=f"t_sb{ih}") for ih in range(2)]
            den_sb = [cst.tile([1, IW], BF16, name=f"den_sb{ih}",
                               tag=f"den_sb{ih}") for ih in range(2)]
            oT_sb = [cst.tile([KDA, IW], F32, name=f"oT_sb{ih}",
                              tag=f"oT_sb{ih}") for ih in range(2)]
            wqkv = [cst.tile([128, KD + KD + KDA], F16, name=f"wqkv{dc}",
                             tag=f"wqkv{dc}") for dc in range(2)]
            wq = [wqkv[dc][:, 0:KD] for dc in range(2)]
            wk = [wqkv[dc][:, KD:2 * KD] for dc in range(2)]
            wv = [wqkv[dc][:, 2 * KD:2 * KD + KDA] for dc in range(2)]
            bqk = cst.tile([KD, 2], F32, tag="bqk")
            bq_c = bqk[:, 0:1]
            bk_c = bqk[:, 1:2]
            bv_r = cst.tile([1, KDA], F16, tag="bv_r")
            ident = cst.tile([128, 128], F32, tag="ident")
            ones_row16 = cst.tile([1, 128], F16, tag="ones_row16")
            ones_row_bf = cst.tile([1, 128], BF16, tag="ones_row_bf")
            ones_col_bf = cst.tile([128, 1], BF16, tag="ones_col_bf")
            mshift_col = cst.tile([128, 1], F32, tag="mshift_col")

            nc.sync.dma_start(ident[:], ident_d.ap())
            nc.vector.memset(ones_row16[:], 1.0)
            nc.vector.memset(ones_row_bf[:], 1.0)
            nc.vector.memset(ones_col_bf[:], 1.0)
            nc.vector.memset(mshift_col[:], -M_SHIFT)

            state = {}
            o_all = cst.tile([128, (HALF // 128) * KD], F32, tag="o_all")
            out_bd = out_d[:].rearrange("(b p) c -> p b c", b=HALF // 128)

            def store_all():
                nc.sync.dma_start(
                    out_bd, o_all[:].rearrange("p (b c) -> p b c",
                                               b=HALF // 128))

            def store_zeros():
                nc.vector.memset(o_all[:], 0.0)
                store_all()

            def emit_lead():
                kT = cst.tile([KD, S], F16, name="kT", tag="kT", bufs=2)
                qT = [cst.tile([KD, IW], F16, name=f"qT{ih}", tag=f"qT{ih}")
                      for ih in range(2)]
                state["kT"], state["qT"] = kT, qT
                for dc in range(2):
                    nc.sync.dma_start(wqkv[dc][:],
                                      wqkv_d[dc * 128:(dc + 1) * 128, :])
                nc.sync.dma_start(bqk[:], bqk_d[:])
                nc.sync.dma_start(bv_r[:], bvr_d[:])
                for hh in range(2):
                    hsl = slice(hh * HALF, (hh + 1) * HALF)
                    for dc in range(2):
                        nc.sync.dma_start(xt[dc][:, hsl],
                                          xT_d[dc * 128:(dc + 1) * 128, hsl])
                emit_K(0)
                emit_Q(0, 0)
                emit_Q(0, 1)
                ps = scp.tile([128, IW], F32, tag="s")
                for q in range(4):
                    nc.tensor.matmul(ps[:, q * KDA:(q + 1) * KDA],
                                     ones_row16[:], bv_r[:],
                                     start=True, stop=True)
                nc.vector.tensor_copy(bvb[:], ps[:, 0:4 * KDA])

            def emit_K(it, eng="dve"):
                sl = slice(it * 512, (it + 1) * 512)
                ps = scp.tile([128, IW], F32, tag="s")
                nc.tensor.matmul(ps[0:KD, 0:512], wk[0][:], xt[0][:, sl],
                                 start=True, stop=False)
                nc.tensor.matmul(ps[0:KD, 0:512], wk[1][:], xt[1][:, sl],
                                 start=False, stop=True)
                if eng == "act":
                    nc.scalar.activation(state["kT"][:, sl], ps[0:KD, 0:512],
                                         IDENT_FN, bias=bk_c[:])
                else:
                    nc.vector.tensor_scalar(state["kT"][:, sl], ps[0:KD, 0:512],
                                            bk_c[:], None, ADD)

            def emit_Q(ih, it):
                sl = slice(ih * IW + it * 512, ih * IW + (it + 1) * 512)
                ps = scp.tile([128, IW], F32, tag="s")
                nc.tensor.matmul(ps[0:KD, 0:512], wq[0][:], xt[0][:, sl],
                                 start=True, stop=False)
                nc.tensor.matmul(ps[0:KD, 0:512], wq[1][:], xt[1][:, sl],
                                 start=False, stop=True)
                nc.vector.tensor_scalar(state["qT"][ih][:, it * 512:(it + 1) * 512],
                                        ps[0:KD, 0:512], bq_c[:], None, ADD)

            def emit_V(g4):
                ps = scp.tile([128, IW], F32, tag="s")
                for q in range(4):
                    jc = g4 * 4 + q
                    sl = slice(jc * 128, (jc + 1) * 128)
                    osl = slice(q * KDA, (q + 1) * KDA)
                    nc.tensor.matmul(ps[:, osl], xt[0][:, sl], wv[0][:],
                                     start=True, stop=False)
                    nc.tensor.matmul(ps[:, osl], xt[1][:, sl], wv[1][:],
                                     start=False, stop=True)
                nc.vector.tensor_tensor(v_all[:, g4 * 4 * KDA:(g4 + 1) * 4 * KDA],
                                        ps[:, 0:4 * KDA], bvb[:], ADD)

            def emit_A(ih, jc):
                p, lo = jc // 2, (jc % 2) * IW
                s_ps = scp.tile([128, IW], F32, tag="s")
                for mt in range(IW // 512):
                    msl = slice(mt * 512, (mt + 1) * 512)
                    nc.tensor.matmul(s_ps[:, mt * 512:(mt + 1) * 512],
                                     state["kT"][:, jc * 128:(jc + 1) * 128],
                                     state["qT"][ih][:, msl],
                                     start=True, stop=True)
                nc.scalar.activation(ep[p][ih][:, lo:lo + IW], s_ps[:], EXP,
                                     bias=mshift_col[:])
                if DEN_MODE[ih] == "pe":
                    den_ps = state["den_ps"]
                    for mt in range(2):
                        msl = slice(mt * 512, (mt + 1) * 512)
                        nc.tensor.matmul(
                            den_ps[0:1, msl], ones_col_bf[:],
                            ep[p][ih][:, lo + mt * 512:lo + (mt + 1) * 512],
                            start=(jc == 0), stop=(jc == NJC - 1))
                elif jc % 2 == 1:
                    if p == 0:
                        state["esum"] = ep[0][ih]
                    else:
                        es = work.tile([128, 2 * IW], BF16, name="esum",
                                       tag="esum", bufs=2)
                        nc.vector.tensor_tensor(es[:], state["esum"][:],
                                                ep[p][ih][:], ADD)
                        state["esum"] = es

            def emit_T(ih, den_ps=None):
                if DEN_MODE[ih] == "pe":
                    if den_ps is None:
                        den_ps = state["den_ps"]
                else:
                    den_ps = pbp.tile([128, IW], F32, tag="pb")
                    es = state["esum"]
                    for mt in range(4):
                        msl = slice((mt % 2) * 512, (mt % 2) * 512 + 512)
                        nc.tensor.matmul(den_ps[0:1, msl], ones_col_bf[:],
                                         es[:, mt * 512:(mt + 1) * 512],
                                         start=(mt < 2), stop=(mt >= 2))
                nc.scalar.activation(den_sb[ih][:], den_ps[0:1, 0:IW],
                                     COPY, scale=1.0 / S)
                t_ps = pbp.tile([128, IW], F32, tag="pb")
                for mt in range(IW // 512):
                    msl = slice(mt * 512, (mt + 1) * 512)
                    nc.tensor.matmul(t_ps[:, msl], ones_row_bf[:],
                                     den_sb[ih][0:1, msl],
                                     start=True, stop=True)
                nc.vector.tensor_copy(t_sb[ih][:], t_ps[:])

            def emit_B(ih, p, oT_ps):
                m = work.tile([128, 2 * IW], BF16, tag="m", bufs=2)
                for q in range(2):
                    qsl = slice(q * IW, (q + 1) * IW)
                    nc.vector.tensor_tensor(m[:, qsl], ep[p][ih][:, qsl],
                                            t_sb[ih][:], IS_GT)
                g = work.tile([128, 2 * IW], BF16, tag="g", bufs=2)
                nc.vector.tensor_tensor(g[:], m[:], ep[p][ih][:], MULT)
                for sub in range(4):
                    osl = slice((sub % 2) * 512, (sub % 2) * 512 + 512)
                    nc.tensor.matmul(oT_ps[0:KDA, osl],
                                     v_sb[2 * p + sub // 2][:],
                                     g[:, sub * 512:(sub + 1) * 512],
                                     start=(p == 0 and sub < 2),
                                     stop=(p == NPAIR - 1 and sub >= 2))

            def emit_tail(ih, it):
                ic = ih * (IW // 128) + it
                isl = slice(it * 128, (it + 1) * 128)
                tr_ps = scp.tile([128, IW], F32, tag="s")
                nc.tensor.transpose(tr_ps[:, 0:KDA], oT_sb[ih][:, isl],
                                    ident[0:KDA, 0:KDA])
                rec = work.tile([128, 1], F32, tag="rec", bufs=2)
                nc.vector.reciprocal(rec[:], tr_ps[:, KD:KDA])
                nc.scalar.activation(o_all[:, ic * KD:(ic + 1) * KD],
                                     tr_ps[:, 0:KD], COPY, scale=rec[:])

            def emit_B1_block(oT_ps):
                """Non-pipelined B1 + tails (last repeat / low-phase path)."""
                for p in range(NPAIR):
                    emit_B(1, p, oT_ps)
                    if p % 2 == 1:
                        emit_tail(0, p // 2)
                nc.scalar.activation(oT_sb[1][:], oT_ps[0:KDA, :], COPY)
                for it in range(IW // 128):
                    emit_tail(1, it)
                store_all()

            for r in range(repeat):
                pipelined = (phase >= 4 and r > 0)
                emit_lead()
                if phase < 2:
                    store_zeros()
                    continue
                if pipelined:
                    emit_T(1, den_ps=state.pop("den1_ps"))
                # A0 window: scores/exp/esum + V proj + deferred B1(r-1)
                oT1_prev = state.get("oT1_ps")
                if DEN_MODE[0] == "pe":
                    state["den_ps"] = dnp.tile([128, IW], F32, name="den_ps",
                                               tag="dn")
                for jc in range(NJC):
                    emit_A(0, jc)
                    if jc % 4 == 1 and jc < 29:
                        emit_K(jc // 4 + 1, eng="act")
                    if jc % 4 == 0:
                        emit_V(jc // 4)
                    if jc in (8, 10):
                        emit_Q(1, (jc - 8) // 2)
                    if pipelined and jc % 2 == 1:
                        emit_B(1, jc // 2, oT1_prev)
                if pipelined:
                    nc.scalar.activation(oT_sb[1][:], oT1_prev[0:KDA, :], COPY)
                if phase < 3:
                    emit_T(0)
                    store_zeros()
                    continue
                # A1 window: scores/exp + B0 masks + deferred tails(r-1).
                # The first 4 score slots are emitted before the t0 chain so
                # the threshold matmuls don't head-block the PE queue.
                den0_ps = state.get("den_ps")
                oT0_ps = pbp.tile([128, IW], F32, name="oT0_ps", tag="pb")
                if DEN_MODE[1] == "pe":
                    state["den_ps"] = dnp.tile([128, IW], F32, name="den_ps",
                                               tag="dn")
                for jc in range(NJC):
                    emit_A(1, jc)
                    if jc == 3:
                        state["den_ps"], den0_ps = den0_ps, state["den_ps"]
                        emit_T(0)
                        state["den_ps"], den0_ps = den0_ps, state["den_ps"]
                    if jc % 2 == 1 and jc >= 5:
                        emit_B(0, (jc - 5) // 2, oT0_ps)
                    if pipelined and jc % 2 == 0 and jc >= 6:
                        ih_t, it_t = divmod((jc - 6) // 2, IW // 128)
                        emit_tail(ih_t, it_t)
                for p in range(NPAIR - 2, NPAIR):
                    emit_B(0, p, oT0_ps)
                if pipelined:
                    for k in range(13, 16):
                        ih_t, it_t = divmod(k, IW // 128)
                        emit_tail(ih_t, it_t)
                nc.scalar.activation(oT_sb[0][:], oT0_ps[0:KDA, :], COPY)
                if pipelined:
                    store_all()
                if phase < 4:
                    emit_T(1)
                    store_zeros()
                    continue
                oT1_ps = pbp.tile([128, IW], F32, name="oT1_ps", tag="pb")
                state["oT1_ps"] = oT1_ps
                if r == repeat - 1:
                    emit_T(1)
                    emit_B1_block(oT1_ps)
                else:
                    state["den1_ps"] = (state["den_ps"]
                                        if DEN_MODE[1] == "pe" else None)

    nc.compile()
    return nc


# ---------------------------------------------------------------------------
# Host side: shard, run on 8 cores, gather.
# ---------------------------------------------------------------------------

_CACHE: dict = {}


def _in_maps(x, Wq, bq, Wk, bk, Wv, bv):
    wqkv = np.zeros((D, KD + KD + KDA), dtype=np.float16)
    wqkv[:, 0:KD] = np.asarray(Wq, dtype=np.float16)
    wqkv[:, KD:2 * KD] = np.asarray(Wk, dtype=np.float16)
    wqkv[:, 2 * KD:2 * KD + KD] = np.asarray(Wv, dtype=np.float16)
    bv_aug = np.zeros((1, KDA), dtype=np.float16)
    bv_aug[0, 0:KD] = np.asarray(bv, dtype=np.float16)
    bv_aug[0, KD] = 1.0
    bqk = np.zeros((KD, 2), dtype=np.float32)
    bqk[:, 0] = np.asarray(bq, np.float32)
    bqk[:, 1] = np.asarray(bk, np.float32)
    maps = []
    for c in range(NCORES):
        b, h = divmod(c, 2)
        xb = np.asarray(x[b], dtype=np.float32)
        # rotate rows so this core's query rows come first, then transpose
        xrot = np.roll(xb, -h * HALF, axis=0)
        maps.append({
            "xT": np.ascontiguousarray(xrot.T).astype(np.float16),
            "Wqkv": wqkv,
            "bqk_col": bqk,
            "bv_aug_row": bv_aug,
        })
    return maps


def get_runner():
    """Build the program once and return (nc, run_fn).

    run_fn(in_maps) -> list of per-core output dicts.  The jitted PJRT
    callable is cached so repeated kernel() calls don't recompile.
    """
    if "runner" in _CACHE:
        return _CACHE["runner"]

    nc = build_program()

    import jax
    from jax.sharding import Mesh, PartitionSpec
    from jax.experimental.shard_map import shard_map
    from concourse import bass2jax
    from concourse import mybir as _mybir

    bass2jax.install_neuronx_cc_hook()

    partition_name = nc.partition_id_tensor.name if nc.partition_id_tensor else None
    in_names, out_names, out_avals = [], [], []
    for alloc in nc.m.functions[0].allocations:
        if not isinstance(alloc, _mybir.MemoryLocationSet):
            continue
        name = alloc.memorylocations[0].name
        if alloc.kind == "ExternalInput":
            if name != partition_name:
                in_names.append(name)
        elif alloc.kind == "ExternalOutput":
            out_names.append(name)
            out_avals.append(jax.core.ShapedArray(
                tuple(alloc.tensor_shape), _mybir.dt.np(alloc.dtype)))
    n_params = len(in_names)
    all_names = in_names + out_names
    if partition_name is not None:
        all_names = all_names + [partition_name]

    def _body(*args):
        operands = list(args)
        if partition_name is not None:
            operands.append(bass2jax.partition_id_tensor())
        outs = bass2jax._bass_exec_p.bind(
            *operands,
            out_avals=tuple(out_avals),
            in_names=tuple(all_names),
            out_names=tuple(out_names),
            lowering_input_output_aliases=(),
            sim_require_finite=False,
            sim_require_nnan=False,
            nc=nc,
        )
        return tuple(outs)

    # Bust any HLO-module-level executable caching when the program changes:
    # the jit module name includes a content hash of the BIR.
    import hashlib
    _body.__name__ = "body_" + hashlib.sha256(nc.to_json_bytes()).hexdigest()[:12]

    devices = jax.devices()[:NCORES]
    mesh = Mesh(np.asarray(devices), ("core",))
    n_outs = len(out_names)
    sharded = jax.jit(shard_map(
        _body, mesh=mesh,
        in_specs=(PartitionSpec("core"),) * (n_params + n_outs),
        out_specs=(PartitionSpec("core"),) * n_outs,
        check_rep=False,
    ), keep_unused=True)

    def run_fn(maps):
        concat_in = [
            np.concatenate([np.asarray(maps[c][nm]) for c in range(NCORES)], axis=0)
            for nm in in_names
        ]
        concat_zero = [
            np.zeros((NCORES * av.shape[0], *av.shape[1:]), av.dtype)
            for av in out_avals
        ]
        outs = sharded(*concat_in, *concat_zero)
        return [
            {nm: np.asarray(outs[i]).reshape(NCORES, *out_avals[i].shape)[c]
             for i, nm in enumerate(out_names)}
            for c in range(NCORES)
        ]

    _CACHE["runner"] = (nc, run_fn, sharded, in_names, out_avals, out_names)
    return _CACHE["runner"]


def kernel(x, Wq, bq, Wk, bk, Wv, bv):
    _, run_fn, *_ = get_runner()
    results = run_fn(_in_maps(x, Wq, bq, Wk, bk, Wv, bv))
    out = np.empty((B, S, KD), dtype=np.float32)
    for c in range(NCORES):
        b, h = divmod(c, 2)
        out[b, h * HALF:(h + 1) * HALF, :] = results[c]["out"]
    return out
